# revision 1
# baseline (speedup 1.0000x reference)
"""GAT (GATConv forward) on 8 trn2 NeuronCores.

Architecture:
  - dst-range sharding: core c owns dst nodes [12500c, 12500(c+1)).
  - On-device xp table build ([N,64] f32 DRAM, replicated per core).
  - Edges per core: blocks = (src-chunk k, dst-window w); window = 128 local dst
    nodes, chunk = 25000 src rows (int16 dma_gather range). Each block padded to
    TPB tiles x 128 slots (uniform SPMD structure). Blocks processed in quads to
    batch small DMAs (sem-count limits).
  - Per block: dma_gather xp rows by src; onehot (DVE is_equal, e-major) and
    transposed onehot (via PE ones-replicate + is_equal vs PSUM); PE matmuls:
    a_dst expand (ohT), exp-weighted scatter (oh) into PSUM; denominators ride
    as payload cols 64:68.
  - P2: rdenom expand (ohT) -> alpha per slot.  P3: out = numer*rdenom + bias.
"""
import numpy as np

import concourse.bass as bass
import concourse.bacc as bacc
import concourse.mybir as mybir
from concourse.tile import TileContext
from concourse import library_config

N = 100000
E = 1600000
CORES = 8
NPC = 12500            # dst nodes per core
WIN = 98               # windows of 128 local nodes (12544 >= 12500)
CHUNKS = 4
CROWS = 25000          # src rows per chunk
H, C = 4, 16
AUX = 4
NEG = 0.2
NBLK = CHUNKS * WIN    # blocks per core (must be divisible by QUAD)
QUAD = 4               # blocks per DMA batch


def build_kernel(TPB, n_win=WIN, n_chunks=CHUNKS, n_p0=None):
    dt = mybir.dt
    nblk = n_chunks * n_win
    assert nblk % QUAD == 0
    spb = TPB * 128                       # slots per block
    half = TPB * 64
    if n_p0 is None:
        n_p0 = (N + 127) // 128           # 782
    n_p0_q = (n_p0 + 3) // 4

    nc = bacc.Bacc(None, target_bir_lowering=False)

    xT = nc.dram_tensor("xT", [128, n_p0_q * 512], dt.float32, kind="ExternalInput")
    x_own = nc.dram_tensor("x_own", [128, ((n_win + 3) // 4) * 512], dt.float32,
                           kind="ExternalInput")
    Wext = nc.dram_tensor("Wext", [128, 72], dt.float32, kind="ExternalInput")
    attsrc = nc.dram_tensor("attsrc", [128, 64], dt.float32, kind="ExternalInput")
    biasin = nc.dram_tensor("biasin", [128, 64], dt.float32, kind="ExternalInput")
    iotarow = nc.dram_tensor("iotarow", [128, 128], dt.float32, kind="ExternalInput")
    iotacol = nc.dram_tensor("iotacol", [128, 1], dt.float32, kind="ExternalInput")
    gidx = nc.dram_tensor("gidx", [nblk // QUAD, 128, QUAD * spb // 16], dt.int16,
                          kind="ExternalInput")
    dstrel_w = nc.dram_tensor("dstrel_w", [nblk // QUAD, 128, QUAD * TPB], dt.float32,
                              kind="ExternalInput")
    dstrel_l = nc.dram_tensor("dstrel_l", [nblk // QUAD, 1, QUAD * spb], dt.float32,
                              kind="ExternalInput")

    table = nc.dram_tensor("table", [n_p0 * 128, 64], dt.float32)  # Internal
    out_o = nc.dram_tensor("out_o", [128, n_win, 64], dt.float32, kind="ExternalOutput")
    alpha_o = nc.dram_tensor("alpha_o", [nblk // QUAD, 128, QUAD * TPB * AUX],
                             dt.float32, kind="ExternalOutput")

    with TileContext(nc) as tc:
        with tc.tile_pool(name="const", bufs=1) as cpool, \
             tc.tile_pool(name="accum", bufs=1) as apool, \
             tc.tile_pool(name="w3", bufs=3) as wpool, \
             tc.tile_pool(name="psA", bufs=2, space="PSUM") as ppA, \
             tc.tile_pool(name="psW", bufs=2, space="PSUM") as ppW, \
             tc.tile_pool(name="psP", bufs=2, space="PSUM") as ppP:
            nc.gpsimd.load_library(library_config.mlp)

            # ---- constants ----
            wext_t = cpool.tile([128, 72], dt.float32, tag="wext")
            nc.sync.dma_start(wext_t[:], Wext[:])
            att_t = cpool.tile([128, 64], dt.float32, tag="att")
            nc.sync.dma_start(att_t[:], attsrc[:])
            bias_t = cpool.tile([128, 64], dt.float32, tag="bias")
            nc.sync.dma_start(bias_t[:], biasin[:])
            iota_row = cpool.tile([128, 128], dt.float32, tag="iotar")
            nc.sync.dma_start(iota_row[:], iotarow[:])
            iota_col = cpool.tile([128, 1], dt.float32, tag="iotac")
            nc.sync.dma_start(iota_col[:], iotacol[:])
            ones_row = cpool.tile([1, 128], dt.float32, tag="ones")
            nc.vector.memset(ones_row[:], 1.0)

            # ---- accumulators ----
            accum = apool.tile([128, n_win, 68], dt.float32, tag="accum")
            nc.vector.memset(accum[:], 0.0)
            adst_t = apool.tile([128, n_win, AUX], dt.float32, tag="adst")
            e_all = apool.tile([128, nblk, TPB * AUX], dt.float32, tag="eall")
            rdall = apool.tile([128, n_win, AUX], dt.float32, tag="rdall")

            # ---- P0: xp table ----
            for q in range(n_p0_q):
                xt_t = wpool.tile([128, 512], dt.float32, tag="p0x")
                nc.sync.dma_start(xt_t[:], xT[:, q * 512:(q + 1) * 512])
                st = wpool.tile([128, 4, 64], dt.float32, tag="p0s")
                for j in range(4):
                    i = q * 4 + j
                    if i >= n_p0:
                        break
                    ps = ppP.tile([128, 64], dt.float32, tag="p0p")
                    nc.tensor.matmul(ps[:], xt_t[:, j * 128:(j + 1) * 128],
                                     wext_t[:, 0:64], start=True, stop=True)
                    nc.scalar.activation(st[:, j, :], ps[:],
                                         mybir.ActivationFunctionType.Copy)
                nj = min(4, n_p0 - q * 4)
                dest = bass.AP(table, q * 512 * 64,
                               [[64, 128], [128 * 64, nj], [1, 64]])
                nc.sync.dma_start(dest, st[:, 0:nj, :])
            # a_dst for own nodes
            for q in range((n_win + 3) // 4):
                xo_t = wpool.tile([128, 512], dt.float32, tag="p0x")
                nc.sync.dma_start(xo_t[:], x_own[:, q * 512:(q + 1) * 512])
                for j in range(4):
                    w = q * 4 + j
                    if w >= n_win:
                        break
                    ps = ppP.tile([128, AUX], dt.float32, tag="p0p")
                    nc.tensor.matmul(ps[:], xo_t[:, j * 128:(j + 1) * 128],
                                     wext_t[:, 68:72], start=True, stop=True)
                    nc.vector.tensor_copy(adst_t[:, w, :], ps[:])

            # ---- P1 ----
            crows = min(CROWS, n_p0 * 128 // n_chunks)
            for bq in range(nblk // QUAD):
                it4 = wpool.tile([128, QUAD * spb // 16], dt.int16, tag="gidx")
                nc.sync.dma_start(it4[:], gidx[bq])
                drw4 = wpool.tile([128, QUAD * TPB], dt.float32, tag="drw")
                nc.sync.dma_start(drw4[:], dstrel_w[bq])
                drl4 = wpool.tile([1, QUAD * spb], dt.float32, tag="drl")
                nc.sync.dma_start(drl4[:], dstrel_l[bq])
                for j in range(QUAD):
                    b = bq * QUAD + j
                    k = b // n_win
                    w = b % n_win
                    tbl = table[k * crows:(k + 1) * crows, :]
                    g = wpool.tile([128, TPB, 64], dt.float32, tag="g")
                    nc.gpsimd.dma_gather(
                        out_ap=g[:], in_ap=tbl,
                        idxs_ap=it4[:, j * (spb // 16):(j + 1) * (spb // 16)],
                        num_idxs=spb, num_idxs_reg=spb, elem_size=64,
                        single_packet=False)
                    drw = drw4[:, j * TPB:(j + 1) * TPB]
                    drl = drl4[:, j * spb:(j + 1) * spb]
                    # s = sum_c g*att  -> [128, TPB, 4]
                    sm = wpool.tile([128, TPB, 64], dt.float32, tag="sm")
                    nc.vector.tensor_tensor(
                        out=sm[:], in0=g[:],
                        in1=bass.AP(att_t.tensor, att_t[:].offset,
                                    [[att_t[:].ap[0][0], 128], [0, TPB], [1, 64]]),
                        op=mybir.AluOpType.mult)
                    s_t = wpool.tile([128, TPB, AUX], dt.float32, tag="s")
                    nc.vector.tensor_reduce(
                        out=s_t[:], in_=sm[:].rearrange("p t (h c) -> p t h c", c=16),
                        axis=mybir.AxisListType.X, op=mybir.AluOpType.add)
                    # onehot  oh[p, t, jj] = (drw[p, t] == jj)
                    oh = wpool.tile([128, TPB, 128], dt.float32, tag="oh")
                    nc.vector.tensor_tensor(
                        out=oh[:],
                        in0=bass.AP(drw4.tensor, drw.offset,
                                    [[drw.ap[0][0], 128], [1, TPB], [0, 128]]),
                        in1=bass.AP(iota_row.tensor, iota_row[:].offset,
                                    [[iota_row[:].ap[0][0], 128], [0, TPB], [1, 128]]),
                        op=mybir.AluOpType.is_equal)
                    # ohT via PE replicate + is_equal vs PSUM
                    psR0 = ppP.tile([128, half], dt.float32, tag="rep")
                    nc.tensor.matmul(psR0[:], ones_row[:], drl[:, 0:half],
                                     start=True, stop=True)
                    psR1 = ppP.tile([128, half], dt.float32, tag="rep")
                    nc.tensor.matmul(psR1[:], ones_row[:], drl[:, half:],
                                     start=True, stop=True)
                    ohT = wpool.tile([128, TPB * 128], dt.float32, tag="ohT")
                    for hh, psRh in ((0, psR0), (1, psR1)):
                        nc.vector.tensor_tensor(
                            out=ohT[:, hh * half:(hh + 1) * half],
                            in0=bass.AP(iota_col.tensor, iota_col[:].offset,
                                        [[iota_col[:].ap[0][0], 128], [0, half]]),
                            in1=psRh[:],
                            op=mybir.AluOpType.is_equal)
                    # a_dst expand
                    psA = ppA.tile([128, TPB * AUX], dt.float32, tag="psA")
                    for t in range(TPB):
                        nc.tensor.matmul(psA[:, t * AUX:(t + 1) * AUX],
                                         ohT[:, t * 128:(t + 1) * 128],
                                         adst_t[:, w, :], start=True, stop=True)
                    # logits -> lrelu -> exp
                    lg = wpool.tile([128, TPB * AUX], dt.float32, tag="lg")
                    nc.vector.tensor_tensor(
                        out=lg[:], in0=s_t[:].rearrange("p t a -> p (t a)"),
                        in1=psA[:], op=mybir.AluOpType.add)
                    # lrelu on DVE: max(x, 0.2*x)
                    lg2 = wpool.tile([128, TPB * AUX], dt.float32, tag="lg2")
                    nc.vector.tensor_scalar_mul(lg2[:], lg[:], NEG)
                    nc.vector.tensor_tensor(out=lg[:], in0=lg[:], in1=lg2[:],
                                            op=mybir.AluOpType.max)
                    nc.scalar.activation(e_all[:, b, :], lg[:],
                                         mybir.ActivationFunctionType.Exp)
                    # payload: cols 0:64 = g * e ; cols 64:68 = e
                    pay = wpool.tile([128, TPB, 68], dt.float32, tag="pay")
                    ea = e_all[:, b, :]
                    nc.vector.tensor_tensor(
                        out=pay[:, :, 0:64], in0=g[:],
                        in1=bass.AP(e_all.tensor, ea.offset,
                                    [[ea.ap[0][0], 128], [AUX, TPB], [1, AUX], [0, 16]]),
                        op=mybir.AluOpType.mult)
                    nc.vector.tensor_copy(
                        pay[:, :, 64:68], ea.rearrange("p (t a) -> p t a", a=AUX))
                    # scatter
                    psW = ppW.tile([128, 68], dt.float32, tag="psW")
                    for t in range(TPB):
                        nc.tensor.matmul(psW[:], oh[:, t, :], pay[:, t, :],
                                         start=(t == 0), stop=(t == TPB - 1))
                    nc.vector.tensor_tensor(out=accum[:, w, :], in0=accum[:, w, :],
                                            in1=psW[:], op=mybir.AluOpType.add)

            # ---- denominators -> reciprocal ----
            dn = apool.tile([128, n_win, AUX], dt.float32, tag="dn")
            nc.vector.tensor_scalar_add(dn[:], accum[:, :, 64:68], 1e-16)
            nc.vector.reciprocal(rdall[:], dn[:])

            # ---- P2: alpha ----
            for bq in range(nblk // QUAD):
                drl4 = wpool.tile([1, QUAD * spb], dt.float32, tag="drl2")
                nc.sync.dma_start(drl4[:], dstrel_l[bq])
                al4 = wpool.tile([128, QUAD * TPB * AUX], dt.float32, tag="al")
                for j in range(QUAD):
                    b = bq * QUAD + j
                    w = b % n_win
                    drl = drl4[:, j * spb:(j + 1) * spb]
                    psR0 = ppP.tile([128, half], dt.float32, tag="rep")
                    nc.tensor.matmul(psR0[:], ones_row[:], drl[:, 0:half],
                                     start=True, stop=True)
                    psR1 = ppP.tile([128, half], dt.float32, tag="rep")
                    nc.tensor.matmul(psR1[:], ones_row[:], drl[:, half:],
                                     start=True, stop=True)
                    ohT = wpool.tile([128, TPB * 128], dt.float32, tag="ohT2")
                    for hh, psRh in ((0, psR0), (1, psR1)):
                        nc.vector.tensor_tensor(
                            out=ohT[:, hh * half:(hh + 1) * half],
                            in0=bass.AP(iota_col.tensor, iota_col[:].offset,
                                        [[iota_col[:].ap[0][0], 128], [0, half]]),
                            in1=psRh[:],
                            op=mybir.AluOpType.is_equal)
                    psA = ppA.tile([128, TPB * AUX], dt.float32, tag="psA")
                    for t in range(TPB):
                        nc.tensor.matmul(psA[:, t * AUX:(t + 1) * AUX],
                                         ohT[:, t * 128:(t + 1) * 128],
                                         rdall[:, w, :], start=True, stop=True)
                    nc.vector.tensor_tensor(
                        out=al4[:, j * TPB * AUX:(j + 1) * TPB * AUX],
                        in0=e_all[:, b, :], in1=psA[:], op=mybir.AluOpType.mult)
                nc.sync.dma_start(alpha_o[bq], al4[:])

            # ---- P3: out ----
            om = apool.tile([128, n_win, 64], dt.float32, tag="om")
            nc.vector.tensor_tensor(
                out=om[:], in0=accum[:, :, 0:64],
                in1=bass.AP(rdall.tensor, rdall[:].offset,
                            [[rdall[:].ap[0][0], 128], [rdall[:].ap[1][0], n_win],
                             [1, AUX], [0, 16]]),
                op=mybir.AluOpType.mult),
            nc.vector.tensor_tensor(
                out=om[:], in0=om[:],
                in1=bass.AP(bias_t.tensor, bias_t[:].offset,
                            [[bias_t[:].ap[0][0], 128], [0, n_win], [1, 64]]),
                op=mybir.AluOpType.add)
            nc.sync.dma_start(out_o[:], om[:])

    nc.compile()
    return nc


def host_prep(x, W, att_src, att_dst, bias, edge_index, TPB):
    src = np.asarray(edge_index[0], dtype=np.int64)
    dst = np.asarray(edge_index[1], dtype=np.int64)
    order = np.argsort(dst, kind="stable")
    dst_s = dst[order]
    src_s = src[order]

    A_src = np.zeros((64, 4), np.float32)
    A_dst = np.zeros((64, 4), np.float32)
    for h in range(H):
        A_src[h * C:(h + 1) * C, h] = np.asarray(att_src)[h]
        A_dst[h * C:(h + 1) * C, h] = np.asarray(att_dst)[h]
    W = np.asarray(W, dtype=np.float32)
    Wext = np.zeros((128, 72), np.float32)
    Wext[:, 0:64] = W
    Wext[:, 64:68] = W @ A_src
    Wext[:, 68:72] = W @ A_dst

    n_p0 = (N + 127) // 128
    n_p0_q = (n_p0 + 3) // 4
    xT = np.zeros((128, n_p0_q * 512), np.float32)
    xT[:, :N] = np.asarray(x, dtype=np.float32).T

    spb = TPB * 128
    in_maps = []
    slot_maps = []
    core_bounds = np.searchsorted(dst_s, np.arange(CORES + 1) * NPC)
    consts = {
        "Wext": Wext,
        "attsrc": np.tile(np.asarray(att_src, np.float32).reshape(1, 64), (128, 1)),
        "biasin": np.tile(np.asarray(bias, np.float32).reshape(1, 64), (128, 1)),
        "iotarow": np.tile(np.arange(128, dtype=np.float32).reshape(1, 128), (128, 1)),
        "iotacol": np.arange(128, dtype=np.float32).reshape(128, 1),
    }
    for c in range(CORES):
        lo, hi = core_bounds[c], core_bounds[c + 1]
        eids = order[lo:hi]
        dstl = dst_s[lo:hi] - c * NPC
        srcg = src_s[lo:hi]
        win = dstl >> 7
        chunk = srcg // CROWS
        blk = chunk * WIN + win
        bo = np.argsort(blk, kind="stable")
        blk_s = blk[bo]
        cnt = np.bincount(blk_s, minlength=NBLK)
        assert cnt.max() <= spb, f"block overflow: {cnt.max()} > {spb}"
        starts = np.arange(NBLK) * spb
        offs_in_blk = np.arange(len(bo)) - np.repeat(
            np.concatenate([[0], np.cumsum(cnt)[:-1]]), cnt)
        slots = starts[blk_s] + offs_in_blk
        S = NBLK * spb
        idx_lin = np.zeros(S, np.int16)
        drel_lin = np.full(S, 999.0, np.float32)
        idx_lin[slots] = (srcg[bo] - (blk_s // WIN) * CROWS).astype(np.int16)
        drel_lin[slots] = (dstl[bo] & 127).astype(np.float32)
        # wrap idx per block then group in quads
        idx_b = idx_lin.reshape(NBLK, spb // 16, 16).transpose(0, 2, 1)  # [B,16,c]
        idx_b = np.tile(idx_b, (1, 8, 1))                                # [B,128,c]
        gidx_q = idx_b.reshape(NBLK // QUAD, QUAD, 128, spb // 16) \
                      .transpose(0, 2, 1, 3).reshape(NBLK // QUAD, 128,
                                                     QUAD * spb // 16)
        drw_b = drel_lin.reshape(NBLK, TPB, 128).transpose(0, 2, 1)      # [B,128,TPB]
        drw_q = drw_b.reshape(NBLK // QUAD, QUAD, 128, TPB) \
                     .transpose(0, 2, 1, 3).reshape(NBLK // QUAD, 128, QUAD * TPB)
        drl_q = drel_lin.reshape(NBLK // QUAD, 1, QUAD * spb)
        n_own_q = (WIN + 3) // 4
        x_own = np.zeros((128, n_own_q * 512), np.float32)
        n0 = c * NPC
        n1 = min(n0 + WIN * 128, N)
        x_own[:, :n1 - n0] = xT[:, n0:n1]
        in_maps.append({
            "xT": xT, "x_own": x_own, **consts,
            "gidx": np.ascontiguousarray(gidx_q),
            "dstrel_w": np.ascontiguousarray(drw_q),
            "dstrel_l": np.ascontiguousarray(drl_q),
        })
        slot_maps.append((eids[bo], slots))
    return in_maps, slot_maps


def compute_tpb(edge_index):
    src = np.asarray(edge_index[0], dtype=np.int64)
    dst = np.asarray(edge_index[1], dtype=np.int64)
    key = (dst // NPC) * NBLK + (src // CROWS) * WIN + ((dst % NPC) >> 7)
    cnt = np.bincount(key, minlength=CORES * NBLK)
    return max(5, int(np.ceil(cnt.max() / 128)))


def run(x, W, att_src, att_dst, bias, edge_index, nc=None, TPB=None):
    from concourse.bass_utils import run_bass_kernel_spmd
    if TPB is None:
        TPB = compute_tpb(edge_index)
    in_maps, slot_maps = host_prep(x, W, att_src, att_dst, bias, edge_index, TPB)
    if nc is None:
        nc = build_kernel(TPB)
    res = run_bass_kernel_spmd(nc, in_maps, core_ids=list(range(CORES)))
    out = np.zeros((N, 64), np.float32)
    alpha = np.zeros((E, 4), np.float32)
    spb = TPB * 128
    for c in range(CORES):
        oo = res.results[c]["out_o"]          # [128, WIN, 64]
        ao = res.results[c]["alpha_o"]        # [NBLK//QUAD, 128, QUAD*TPB*4]
        n0 = c * NPC
        nn = min(NPC, N - n0)
        oo_nodes = oo.transpose(1, 0, 2).reshape(WIN * 128, 64)
        out[n0:n0 + nn] = oo_nodes[:nn]
        eids, slots = slot_maps[c]
        b = slots // spb
        i = slots % spb
        av = ao[b // QUAD, i % 128]           # [n, QUAD*TPB*4]
        av = av.reshape(len(slots), QUAD, TPB, 4)[
            np.arange(len(slots)), b % QUAD, i // 128]
        alpha[eids] = av
    return out, np.asarray(edge_index), alpha


_NC_CACHE = {}


def kernel(x, W, att_src, att_dst, bias, edge_index):
    """Full-input GAT forward on 8 NeuronCores; returns (out, edge_index, alpha)."""
    TPB = compute_tpb(edge_index)
    nc = _NC_CACHE.get(TPB)
    if nc is None:
        nc = build_kernel(TPB)
        _NC_CACHE[TPB] = nc
    return run(x, W, att_src, att_dst, bias, edge_index, nc=nc, TPB=TPB)
